# revision 12
# baseline (speedup 1.0000x reference)
"""Trainium2 Bass kernel for nn_CrossAttention_59871844106349.

Cross-attention over flattened 16^3 spatial grid, per batch:
  q = wq@x+bq  [N,32];  k = wk@x+bk  [32,N];  v = wv@x+bv  [256,N]
  out = v @ softmax(q@k, axis=-1)^T + x      (N = 4096, B = 4)

Sharding: 8 cores = (batch b, query-half h).  Each core receives the full
(rotated) batch image xf [256, 4096] bf16 with its 2048 query columns
rotated to the front, computes K/V for all 4096 keys and the attention
output for its 2048 queries, writes out [256, 2048] bf16.

On-core algorithm (layouts chosen so no transposes are ever needed):
  - K4 [128, 4096]: 4 replicas of k [32, N] stacked in partition groups of
    32 (for 4-way row-packed score matmuls).  Computed with 4x-replicated
    weights wk4 [256, 128] so one matmul pass produces all replicas.
  - QT4 [128, 2048]: same for q^T.
  - vt [128, 32*256]: v transposed (key index on partitions), fp8e4 when
    ES_FP8 else bf16, V bias deferred into the residual (sums to bv after
    softmax-normalization).
  - Scores computed TRANSPOSED: S^T[m, n] = sum_d K[d,m] QT[d,n] via 4
    concurrent tile_position row-group matmuls into s_ps [128, 2048].
  - exp on ScalarE in two [128, 1024] halves (lets next step's score
    matmuls overwrite the first half of s_ps while the second half is
    still being read -> ScalarE never idles).  ES_FP8: out fp8e5 with a
    global bias shift (softmax is shift-invariant; shift keeps es in fp8
    range).  Else bf16, no shift.
  - A*V: PSUM accumulation over key tiles.  ES_FP8: DoubleRow fp8 matmuls
    (256-key contraction per instruction).  Row sums via ones-weight
    matmuls (DoubleRow [1,512] accumulator in fp8 mode; 4-way col-packed
    tile_position [128,512] replica accumulator in bf16 mode).
  - normalize: broadcast/total via one matmul, reciprocal_approx_accurate,
    multiply + residual add on VectorE, DMA out bf16.
"""

import numpy as np
import ml_dtypes

_B, _C, _CQK, _N = 4, 256, 32, 4096
_NQ = _N // 2  # queries per core
_NCORES = 8
_BF16 = ml_dtypes.bfloat16

ES_FP8 = False       # es fp8e5 + vt fp8e4 + DoubleRow A*V matmuls
EXP_SHIFT = 14.5    # global score shift in fp8 mode (softmax-invariant)
EXP_SPLITS = 2      # exp instructions per [128, 2048] score tile

_RT: dict = {}


def _ensure_imports():
    try:
        import concourse.bass  # noqa: F401
    except ImportError:
        import sys

        for p in ("/opt/trn_rl_repo", "/root/.axon_site/_ro/trn_rl_repo"):
            if p not in sys.path:
                sys.path.append(p)
        import concourse.bass  # noqa: F401


def _build_nc(repeat=1, es_fp8=None, exp_splits=None):
    """Build and bacc-compile the single-core Bass program (SPMD across 8).

    repeat>1 wraps the entire kernel body in a hardware For-loop; used only
    for timing (amortizes the per-dispatch overhead over many iterations).
    """
    import concourse.bass as bass
    import concourse.tile as tile
    from concourse import bacc, mybir

    if es_fp8 is None:
        es_fp8 = ES_FP8
    if exp_splits is None:
        exp_splits = EXP_SPLITS

    f32 = mybir.dt.float32
    f32r = mybir.dt.float32r
    bf16 = mybir.dt.bfloat16
    fp8e4 = mybir.dt.float8e4
    fp8e5 = mybir.dt.float8e5
    EXP = mybir.ActivationFunctionType.Exp
    DR = mybir.MatmulPerfMode.DoubleRow

    es_dt = fp8e5 if es_fp8 else bf16
    vt_dt = fp8e4 if es_fp8 else bf16
    shift = EXP_SHIFT if es_fp8 else 0.0

    nc = bacc.Bacc("TRN2", target_bir_lowering=False, debug=False)

    xf_d = nc.dram_tensor("xf", [_C, _N], bf16, kind="ExternalInput").ap()
    wq4_d = nc.dram_tensor("wq4", [_C, 128], bf16, kind="ExternalInput").ap()
    wk4_d = nc.dram_tensor("wk4", [_C, 128], bf16, kind="ExternalInput").ap()
    wvT_d = nc.dram_tensor("wvT", [_C, _C], bf16, kind="ExternalInput").ap()
    bq4_d = nc.dram_tensor("bq4", [128, 1], f32, kind="ExternalInput").ap()
    bk4_d = nc.dram_tensor("bk4", [128, 1], f32, kind="ExternalInput").ap()
    bv2_d = nc.dram_tensor("bv2", [128, 2], f32, kind="ExternalInput").ap()
    sel4_d = nc.dram_tensor("sel4", [128, 128], bf16, kind="ExternalInput").ap()
    out_d = nc.dram_tensor("out", [_C, _NQ], bf16, kind="ExternalOutput").ap()

    NT_M = _N // 128   # 32 key tiles
    NMG = NT_M // 4    # 8 groups of 4 key tiles (one [128,2048] score tile)
    NCH_Q = _NQ // 512  # 4 query chunks

    with tile.TileContext(nc) as tc:
      import contextlib
      loop_cm = tc.For_i(0, repeat, 1) if repeat > 1 else contextlib.nullcontext()
      with loop_cm:
        with tc.tile_pool(name="persist", bufs=1) as persist, \
             tc.tile_pool(name="expp", bufs=3) as expp, \
             tc.tile_pool(name="outp", bufs=2) as outp:

            # ---------------- load inputs ----------------
            # small weight/bias DMAs first so they don't queue behind xf
            wq4 = [persist.tile([128, 128], bf16, tag=f"wq4{t}", name=f"wq4{t}") for t in range(2)]
            wk4 = [persist.tile([128, 128], bf16, tag=f"wk4{t}", name=f"wk4{t}") for t in range(2)]
            wvT = [persist.tile([128, _C], bf16, tag=f"wvT{t}", name=f"wvT{t}") for t in range(2)]
            for t in range(2):
                nc.sync.dma_start(out=wq4[t], in_=wq4_d[t * 128:(t + 1) * 128, :])
                nc.sync.dma_start(out=wk4[t], in_=wk4_d[t * 128:(t + 1) * 128, :])
                nc.sync.dma_start(out=wvT[t], in_=wvT_d[t * 128:(t + 1) * 128, :])

            bq4 = persist.tile([128, 1], f32, tag="bq4")
            bk4 = persist.tile([128, 1], f32, tag="bk4")
            bv2 = persist.tile([128, 2], f32, tag="bv2")
            sel4 = persist.tile([128, 128], bf16, tag="sel4")
            nc.sync.dma_start(out=bq4, in_=bq4_d)
            nc.sync.dma_start(out=bk4, in_=bk4_d)
            nc.sync.dma_start(out=bv2, in_=bv2_d)
            if not es_fp8:
                nc.sync.dma_start(out=sel4, in_=sel4_d)

            # rsum DoubleRow weights: [128, 2, 1] ones with 16B pair stride
            # (s3_lw dual-fp8 LDWEIGHTS requires 16B-aligned outer step)
            ones2 = persist.tile([128, 32], es_dt, tag="ones2")
            nc.vector.memset(ones2, 1.0)
            ones_row = persist.tile([1, 128], bf16, tag="ones_row")
            nc.vector.memset(ones_row, 1.0)
            nshift = persist.tile([128, 1], f32, tag="nshift")
            nc.vector.memset(nshift, -shift)

            # xf loaded in 512KB chunks so projections can start early
            xfb = [persist.tile([128, _N], bf16, tag=f"xfb{t}", name=f"xfb{t}") for t in range(2)]
            for ch in range(4):
                sl = slice(1024 * ch, 1024 * (ch + 1))
                for t in range(2):
                    nc.sync.dma_start(out=xfb[t][:, sl], in_=xf_d[t * 128:(t + 1) * 128, sl])

            # residual-with-bias: x + bv[c] (V bias deferred through the
            # attention: sum_m a[m,n]*bv[c]/rowsum = bv[c])
            xfbv = [persist.tile([128, _NQ], bf16, tag=f"xfbv{t}", name=f"xfbv{t}")
                    for t in range(2)]
            for t in range(2):
                nc.vector.tensor_scalar_add(
                    out=xfbv[t], in0=xfb[t][:, 0:_NQ], scalar1=bv2[:, t:t + 1])

            # ---------------- projections ----------------
            K4 = persist.tile([128, _N], bf16, tag="K4")
            QT4 = persist.tile([128, _NQ], bf16, tag="QT4")
            vt = persist.tile([128, NT_M * _C], vt_dt, tag="vt")

            with tc.tile_pool(name="ps_proj", bufs=4, space="PSUM") as ps_proj:
                def kq_proj(ch, w4, bias, dst):
                    # one 512-col chunk of the 4x-replicated [32,N] projection
                    ps = ps_proj.tile([128, 512], f32, tag="pp", name="pp")
                    for cp in range(2):
                        nc.tensor.matmul(
                            ps,
                            lhsT=w4[cp],
                            rhs=xfb[cp][:, 512 * ch:512 * (ch + 1)],
                            start=(cp == 0), stop=(cp == 1),
                        )
                    nc.vector.tensor_scalar_add(
                        out=dst[:, 512 * ch:512 * (ch + 1)], in0=ps, scalar1=bias)

                def v_proj2(nt2):
                    # two key tiles' VT [128, 256] slabs -> one [128,512] bank
                    ps = ps_proj.tile([128, 512], f32, tag="ppv", name="ppv")
                    for j in range(2):
                        nt = 2 * nt2 + j
                        for cp in range(2):
                            nc.tensor.matmul(
                                ps[:, 256 * j:256 * (j + 1)],
                                lhsT=xfb[cp][:, 128 * nt:128 * (nt + 1)],
                                rhs=wvT[cp],
                                start=(cp == 0), stop=(cp == 1),
                            )
                    nc.vector.tensor_copy(
                        out=vt[:, 512 * nt2:512 * (nt2 + 1)], in_=ps)

                # interleaved: each 512-wide slice of x feeds its K, Q and V
                # projections while the next slice's DMA is in flight
                for ch in range(_N // 512):
                    kq_proj(ch, wk4, bk4, K4)
                    if ch < NCH_Q:
                        kq_proj(ch, wq4, bq4, QT4)
                    for nt2 in range(2 * ch, 2 * (ch + 1)):
                        v_proj2(nt2)

            # ---------------- attention main loop ----------------
            with tc.tile_pool(name="ps_s", bufs=1, space="PSUM") as ps_s, \
                 tc.tile_pool(name="ps_o", bufs=1, space="PSUM") as ps_o, \
                 tc.tile_pool(name="ps_r", bufs=1, space="PSUM") as ps_r, \
                 tc.tile_pool(name="ps_b", bufs=1, space="PSUM") as ps_b:

                def vt_pair_ap(mp, ct):
                    # lhsT [128, 2, 128] for DoubleRow: key tiles (2mp, 2mp+1)
                    sl = vt[:, 512 * mp + 128 * ct:512 * mp + 128 * ct + 384]
                    return bass.AP(
                        tensor=sl.tensor, offset=sl.offset,
                        ap=[list(sl.ap[0]), [256, 2], [1, 128]])

                def emit_av_half(es, mg, half, out_ps, rs_ps):
                    """A*V + row-sum matmuls for one half of an exp group."""
                    if es_fp8:
                        p = half  # pair index within the es tile
                        mp = 2 * mg + p  # global key-tile-pair index
                        rhs = es[:, 1024 * p:1024 * (p + 1)].rearrange(
                            "p (a b) -> p a b", a=2)
                        for ct in range(2):
                            nc.tensor.matmul(
                                out_ps[:, 512 * ct:512 * (ct + 1)],
                                lhsT=vt_pair_ap(mp, ct),
                                rhs=rhs,
                                start=(mp == 0), stop=(mp == NT_M // 2 - 1),
                                perf_mode=DR,
                                skip_group_check=True,
                            )
                        osl = ones2[:, 0:32]
                        ones_dr = bass.AP(
                            tensor=osl.tensor, offset=osl.offset,
                            ap=[list(osl.ap[0]), [16, 2], [1, 1]])
                        nc.tensor.matmul(
                            rs_ps,
                            lhsT=ones_dr,
                            rhs=rhs,
                            start=(mp == 0), stop=(mp == NT_M // 2 - 1),
                            perf_mode=DR,
                            skip_group_check=True,
                        )
                    else:
                        for g in (2 * half, 2 * half + 1):
                            mt = 4 * mg + g
                            for ct in range(2):
                                nc.tensor.matmul(
                                    out_ps[:, 512 * ct:512 * (ct + 1)],
                                    lhsT=vt[:, _C * mt + 128 * ct:_C * mt + 128 * (ct + 1)],
                                    rhs=es[:, 512 * g:512 * (g + 1)],
                                    start=(mt == 0), stop=(mt == NT_M - 1),
                                    skip_group_check=True,
                                )
                            # col-packed row-sum replicas [32, 512] at group g
                            # (each col group accumulates once per mg: its
                            # start/stop must key on mg, not mt)
                            nc.tensor.matmul(
                                rs_ps[32 * g:32 * (g + 1), :],
                                lhsT=ones32,
                                rhs=es[:, 512 * g:512 * (g + 1)],
                                start=(mg == 0), stop=(mg == NMG - 1),
                                tile_position=(0, 32 * g),
                                skip_group_check=True,
                            )

                if not es_fp8:
                    ones32 = persist.tile([128, 32], bf16, tag="ones32")
                    nc.vector.memset(ones32, 1.0)

                def finalize(out_ps, rs_ps, qc):
                    """normalize + residual + store for one finished chunk"""
                    bc_ps = ps_b.tile([128, 512], f32, tag="bc_ps", name="bc_ps")
                    if es_fp8:
                        # rs_ps [1, 512] totals -> recip -> broadcast matmul
                        recip = outp.tile([1, 512], f32, tag="recip", name="recip")
                        rscr = outp.tile([1, 512], f32, tag="rscr", name="rscr")
                        nc.vector.reciprocal_approx_accurate(
                            out=recip, in_=rs_ps, scratch=rscr)
                        recb = outp.tile([1, 512], bf16, tag="recb", name="recb")
                        nc.vector.tensor_copy(out=recb, in_=recip)
                        nc.tensor.matmul(
                            bc_ps, lhsT=ones_row, rhs=recb,
                            start=True, stop=True)
                        bc = outp.tile([128, 512], f32, tag="bc", name="bc")
                        nc.vector.tensor_copy(out=bc, in_=bc_ps)
                    else:
                        # rs_ps [128,512] group replicas -> total via sel4
                        rs4 = outp.tile([128, 512], bf16, tag="rs4", name="rs4")
                        nc.vector.tensor_copy(out=rs4, in_=rs_ps)
                        nc.tensor.matmul(bc_ps, lhsT=sel4, rhs=rs4,
                                         start=True, stop=True)
                        bc = outp.tile([128, 512], f32, tag="bc", name="bc")
                        bscr = outp.tile([128, 512], f32, tag="bscr", name="bscr")
                        nc.vector.reciprocal_approx_accurate(
                            out=bc, in_=bc_ps, scratch=bscr)
                    for ct in range(2):
                        o1 = outp.tile([128, 512], bf16, tag="o1", name="o1")
                        o2 = outp.tile([128, 512], bf16, tag="o2", name="o2")
                        nc.vector.tensor_mul(o1, out_ps[:, 512 * ct:512 * (ct + 1)], bc)
                        nc.vector.tensor_add(o2, o1, xfbv[ct][:, 512 * qc:512 * (qc + 1)])
                        nc.sync.dma_start(
                            out=out_d[128 * ct:128 * (ct + 1), 512 * qc:512 * (qc + 1)],
                            in_=o2,
                        )

                # Pipeline: the A*V for exp half h of step i is emitted just
                # before the same-half score matmuls of step i+1, so the PE
                # has work while the ScalarE runs exp, and score matmuls for
                # a half of s_ps wait only on that half's exp (subtile deps).
                rs_shape = [1, 512] if es_fp8 else [128, 512]
                # one persistent score tile: all pipelining via subtile deps
                s_ps = ps_s.tile([128, 2048], f32, tag="s_ps", name="s_ps")
                pending = None   # (es, mg, accum) awaiting A*V emission
                fin = None       # (out_ps, rs_ps, qc) awaiting finalize
                for qc in range(NCH_Q):
                    cur = None
                    for mg in range(NMG):
                        for half in range(2):
                            if pending is not None:
                                emit_av_half(pending[0], pending[1], half,
                                             *pending[2])
                            for g in (2 * half, 2 * half + 1):
                                mt = 4 * mg + g
                                nc.tensor.matmul(
                                    s_ps[:, 512 * g:512 * (g + 1)],
                                    lhsT=K4[32 * g:32 * (g + 1), 128 * mt:128 * (mt + 1)],
                                    rhs=QT4[32 * g:32 * (g + 1), 512 * qc:512 * (qc + 1)],
                                    start=True, stop=True,
                                    tile_position=(32 * g, 0),
                                )
                        es = expp.tile([128, 2048], es_dt, tag="es", name="es")
                        if exp_splits == 1:
                            nc.scalar.activation(out=es, in_=s_ps, func=EXP,
                                                 bias=nshift)
                        else:
                            for h in range(2):
                                nc.scalar.activation(
                                    out=es[:, 1024 * h:1024 * (h + 1)],
                                    in_=s_ps[:, 1024 * h:1024 * (h + 1)],
                                    func=EXP, bias=nshift)
                        if fin is not None:
                            finalize(*fin)
                            fin = None
                        if cur is None:
                            # allocated only after the previous chunk's
                            # finalize is emitted (bufs=1 bank reuse)
                            cur = (ps_o.tile([128, 1024], f32,
                                             tag="out_ps", name="out_ps"),
                                   ps_r.tile(rs_shape, f32,
                                             tag="rs_ps", name="rs_ps"))
                        pending = (es, mg, cur)
                    # tail: emit the last group's A*V now; finalize next chunk
                    for half in range(2):
                        emit_av_half(pending[0], pending[1], half, *pending[2])
                    pending = None
                    fin = (cur[0], cur[1], qc)
                finalize(*fin)

    nc.compile()
    return nc


def _make_runner(nc):
    """Cached jitted SPMD executor, mirroring bass2jax.run_bass_via_pjrt."""
    import jax
    from jax.experimental.shard_map import shard_map
    from jax.sharding import Mesh, PartitionSpec
    from concourse import mybir
    from concourse.bass2jax import (
        _bass_exec_p,
        install_neuronx_cc_hook,
        partition_id_tensor,
    )

    install_neuronx_cc_hook()

    partition_name = (
        nc.partition_id_tensor.name if nc.partition_id_tensor else None)
    in_names, out_names, out_avals = [], [], []
    for alloc in nc.m.functions[0].allocations:
        if not isinstance(alloc, mybir.MemoryLocationSet):
            continue
        name = alloc.memorylocations[0].name
        if alloc.kind == "ExternalInput":
            if name != partition_name:
                in_names.append(name)
        elif alloc.kind == "ExternalOutput":
            out_names.append(name)
            out_avals.append(
                jax.core.ShapedArray(tuple(alloc.tensor_shape),
                                     mybir.dt.np(alloc.dtype)))
    n_params = len(in_names)
    all_in_names = tuple(in_names + out_names)
    if partition_name is not None:
        all_in_names = all_in_names + (partition_name,)

    def _body(*args):
        operands = list(args)
        if partition_name is not None:
            operands.append(partition_id_tensor())
        outs = _bass_exec_p.bind(
            *operands,
            out_avals=tuple(out_avals),
            in_names=all_in_names,
            out_names=tuple(out_names),
            lowering_input_output_aliases=(),
            sim_require_finite=True,
            sim_require_nnan=True,
            nc=nc,
        )
        return tuple(outs)

    devices = jax.devices()[:_NCORES]
    assert len(devices) == _NCORES
    mesh = Mesh(np.asarray(devices), ("core",))
    n_outs = len(out_names)
    in_specs = (PartitionSpec("core"),) * (n_params + n_outs)
    out_specs = (PartitionSpec("core"),) * n_outs
    donate = tuple(range(n_params, n_params + n_outs))
    sharded = jax.jit(
        shard_map(_body, mesh=mesh, in_specs=in_specs, out_specs=out_specs,
                  check_rep=False),
        donate_argnums=donate, keep_unused=True)
    return {
        "fn": sharded,
        "in_names": in_names,
        "out_names": out_names,
        "out_avals": out_avals,
    }


def _get_runtime():
    if "runner" not in _RT:
        _ensure_imports()
        nc = _build_nc()
        _RT["nc"] = nc
        _RT["runner"] = _make_runner(nc)
    return _RT["runner"]


def _core_inputs(x, wq, bq, wk, bk, wv, bv):
    """Build the 8 per-core input dicts (host-side shard)."""
    x = np.ascontiguousarray(np.asarray(x, dtype=np.float32))
    wq = np.asarray(wq, dtype=np.float32)
    bq = np.asarray(bq, dtype=np.float32)
    wk = np.asarray(wk, dtype=np.float32)
    bk = np.asarray(bk, dtype=np.float32)
    wv = np.asarray(wv, dtype=np.float32)
    bv = np.asarray(bv, dtype=np.float32)

    wq4 = np.ascontiguousarray(np.tile(wq.T, (1, 4))).astype(_BF16)
    wk4 = np.ascontiguousarray(np.tile(wk.T, (1, 4))).astype(_BF16)
    wvT = np.ascontiguousarray(wv.T).astype(_BF16)
    bq4 = np.ascontiguousarray(np.tile(bq, 4).reshape(128, 1))
    bk4 = np.ascontiguousarray(np.tile(bk, 4).reshape(128, 1))
    bv2 = np.ascontiguousarray(bv.reshape(2, 128).T)
    sel4 = np.zeros((128, 128), dtype=_BF16)
    sel4[[0, 32, 64, 96], :] = 1

    in_maps = []
    for c in range(_NCORES):
        b, h = divmod(c, 2)
        xb = x[b].reshape(_C, _N)
        if h:
            xrot = np.roll(xb, -_NQ, axis=1)
        else:
            xrot = xb
        in_maps.append({
            "xf": np.ascontiguousarray(xrot).astype(_BF16),
            "wq4": wq4, "wk4": wk4, "wvT": wvT,
            "bq4": bq4, "bk4": bk4, "bv2": bv2, "sel4": sel4,
        })
    return in_maps


def run_cores(in_maps):
    """Execute the SPMD kernel; returns list of per-core output dicts."""
    r = _get_runtime()
    fn, in_names, out_names, out_avals = (
        r["fn"], r["in_names"], r["out_names"], r["out_avals"])
    per_core = [[np.asarray(m[n]) for n in in_names] for m in in_maps]
    concat_in = [
        np.concatenate([per_core[c][i] for c in range(_NCORES)], axis=0)
        for i in range(len(in_names))
    ]
    concat_zeros = [
        np.zeros((_NCORES * a.shape[0], *a.shape[1:]), a.dtype)
        for a in out_avals
    ]
    out_arrs = fn(*concat_in, *concat_zeros)
    return [
        {
            name: np.asarray(out_arrs[i]).reshape(_NCORES, *out_avals[i].shape)[c]
            for i, name in enumerate(out_names)
        }
        for c in range(_NCORES)
    ]


def kernel(x, wq, bq, wk, bk, wv, bv):
    x = np.asarray(x, dtype=np.float32)
    B, C, D, H, W = x.shape
    assert (B, C, D * H * W) == (_B, _C, _N)

    in_maps = _core_inputs(x, wq, bq, wk, bk, wv, bv)
    results = run_cores(in_maps)

    out = np.empty((_B, _C, _N), dtype=np.float32)
    for c in range(_NCORES):
        b, h = divmod(c, 2)
        out[b][:, h * _NQ:(h + 1) * _NQ] = results[c]["out"]
    return out.reshape(B, C, D, H, W)


# revision 18
# speedup vs baseline: 1.7112x; 1.7112x over previous
"""Trainium2 Bass kernel for nn_CrossAttention_59871844106349.

Cross-attention over flattened 16^3 spatial grid, per batch:
  q = wq@x+bq  [N,32];  k = wk@x+bk  [32,N];  v = wv@x+bv  [256,N]
  out = v @ softmax(q@k, axis=-1)^T + x      (N = 4096, B = 4)

Sharding: 8 cores = (batch b, query-half h).  Each core receives the full
(rotated) batch image xf [256, 4096] bf16 with its 2048 query columns
rotated to the front, computes K/V for all 4096 keys and the attention
output for its 2048 queries, writes out [256, 2048] bf16.

On-core algorithm (layouts chosen so no transposes are ever needed):
  - K4 [128, 4096]: 4 replicas of k [32, N] stacked in partition groups of
    32 (for 4-way row-packed score matmuls).  Computed with 4x-replicated
    weights wk4 [256, 128] so one matmul pass produces all replicas.
  - QT4 [128, 2048]: same for q^T.
  - vt [128, 32*256]: v transposed (key index on partitions), fp8e4 when
    ES_FP8 else bf16, V bias deferred into the residual (sums to bv after
    softmax-normalization).
  - Scores computed TRANSPOSED: S^T[m, n] = sum_d K[d,m] QT[d,n] via 4
    concurrent tile_position row-group matmuls into s_ps [128, 2048].
  - exp on ScalarE in two [128, 1024] halves (lets next step's score
    matmuls overwrite the first half of s_ps while the second half is
    still being read -> ScalarE never idles).  ES_FP8: out fp8e5 with a
    global bias shift (softmax is shift-invariant; shift keeps es in fp8
    range).  Else bf16, no shift.
  - A*V: PSUM accumulation over key tiles.  ES_FP8: DoubleRow fp8 matmuls
    (256-key contraction per instruction).  Row sums via ones-weight
    matmuls (DoubleRow [1,512] accumulator in fp8 mode; 4-way col-packed
    tile_position [128,512] replica accumulator in bf16 mode).
  - normalize: broadcast/total via one matmul, reciprocal_approx_accurate,
    multiply + residual add on VectorE, DMA out bf16.
"""

import numpy as np
import ml_dtypes

_B, _C, _CQK, _N = 4, 256, 32, 4096
_NQ = _N // 2  # queries per core
_NCORES = 8
_BF16 = ml_dtypes.bfloat16

ES_FP8 = True       # es fp8e5 + vt fp8e4 + DoubleRow A*V matmuls
EXP_SHIFT = 15.5    # global score shift in fp8 mode (softmax-invariant)
EXP_SPLITS = 2      # exp instructions per [128, 2048] score tile

_RT: dict = {}


def _ensure_imports():
    try:
        import concourse.bass  # noqa: F401
    except ImportError:
        import sys

        for p in ("/opt/trn_rl_repo", "/root/.axon_site/_ro/trn_rl_repo"):
            if p not in sys.path:
                sys.path.append(p)
        import concourse.bass  # noqa: F401


def _build_nc(repeat=1, es_fp8=None, exp_splits=None):
    """Build and bacc-compile the single-core Bass program (SPMD across 8).

    repeat>1 wraps the entire kernel body in a hardware For-loop; used only
    for timing (amortizes the per-dispatch overhead over many iterations).
    """
    import concourse.bass as bass
    import concourse.tile as tile
    from concourse import bacc, mybir

    if es_fp8 is None:
        es_fp8 = ES_FP8
    if exp_splits is None:
        exp_splits = EXP_SPLITS

    f32 = mybir.dt.float32
    f32r = mybir.dt.float32r
    bf16 = mybir.dt.bfloat16
    fp8e4 = mybir.dt.float8e4
    fp8e5 = mybir.dt.float8e5
    EXP = mybir.ActivationFunctionType.Exp
    IDENT = mybir.ActivationFunctionType.Identity
    COPY = mybir.ActivationFunctionType.Copy
    DR = mybir.MatmulPerfMode.DoubleRow

    es_dt = fp8e5 if es_fp8 else bf16
    vt_dt = fp8e4 if es_fp8 else bf16
    shift = EXP_SHIFT if es_fp8 else 0.0

    nc = bacc.Bacc("TRN2", target_bir_lowering=False, debug=False)

    xf_d = nc.dram_tensor("xf", [_C, _N], bf16, kind="ExternalInput").ap()
    wkq4_d = nc.dram_tensor("wkq4", [_C, 256], bf16, kind="ExternalInput").ap()
    wvT_d = nc.dram_tensor("wvT", [_C, _C], bf16, kind="ExternalInput").ap()
    bias4_d = nc.dram_tensor("bias4", [128, 4], f32, kind="ExternalInput").ap()
    sel4_d = nc.dram_tensor("sel4", [128, 128], bf16, kind="ExternalInput").ap()
    out_d = nc.dram_tensor("out", [_C, _NQ], bf16, kind="ExternalOutput").ap()

    NT_M = _N // 128   # 32 key tiles
    NMG = NT_M // 4    # 8 groups of 4 key tiles (one [128,2048] score tile)
    NCH_Q = _NQ // 512  # 4 query chunks

    with tile.TileContext(nc) as tc:
      import contextlib
      loop_cm = tc.For_i(0, repeat, 1) if repeat > 1 else contextlib.nullcontext()
      with loop_cm:
        with tc.tile_pool(name="persist", bufs=1) as persist, \
             tc.tile_pool(name="expp", bufs=3) as expp, \
             tc.tile_pool(name="outp", bufs=2) as outp:

            # ---------------- load inputs ----------------
            # first xf chunk and the K/Q weights+biases are issued first so
            # the first projection can start as early as possible
            wkq4 = [persist.tile([128, 256], bf16, tag=f"wkq4{t}", name=f"wkq4{t}") for t in range(2)]
            wvT = [persist.tile([128, _C], bf16, tag=f"wvT{t}", name=f"wvT{t}") for t in range(2)]
            bias4 = persist.tile([128, 4], f32, tag="bias4")
            sel4 = persist.tile([128, 128], bf16, tag="sel4")
            xfb = [persist.tile([128, _N], bf16, tag=f"xfb{t}", name=f"xfb{t}") for t in range(2)]

            for t in range(2):
                nc.sync.dma_start(out=xfb[t][:, 0:1024], in_=xf_d[t * 128:(t + 1) * 128, 0:1024])
            for t in range(2):
                nc.sync.dma_start(out=wkq4[t], in_=wkq4_d[t * 128:(t + 1) * 128, :])
            nc.sync.dma_start(out=bias4, in_=bias4_d)
            for t in range(2):
                nc.sync.dma_start(out=wvT[t], in_=wvT_d[t * 128:(t + 1) * 128, :])
            if not es_fp8:
                nc.sync.dma_start(out=sel4, in_=sel4_d)
            for ch in range(1, 4):
                sl = slice(1024 * ch, 1024 * (ch + 1))
                for t in range(2):
                    nc.sync.dma_start(out=xfb[t][:, sl], in_=xf_d[t * 128:(t + 1) * 128, sl])

            wk4 = [wkq4[t][:, 0:128] for t in range(2)]
            wq4 = [wkq4[t][:, 128:256] for t in range(2)]
            bk4 = bias4[:, 1:2]
            bq4 = bias4[:, 0:1]
            bv2 = bias4[:, 2:4]

            # rsum DoubleRow weights: [128, 2, 1] ones with 16B pair stride
            # (s3_lw dual-fp8 LDWEIGHTS requires 16B-aligned outer step)
            ones2 = persist.tile([128, 32], es_dt, tag="ones2")
            nc.vector.memset(ones2, 1.0)
            ones_row = persist.tile([1, 128], bf16, tag="ones_row")
            nc.vector.memset(ones_row, 1.0)
            nshift = persist.tile([128, 1], f32, tag="nshift")
            nc.vector.memset(nshift, -shift)

            # residual-with-bias: x + bv[c] (V bias deferred through the
            # attention: sum_m a[m,n]*bv[c]/rowsum = bv[c])
            xfbv = [persist.tile([128, _NQ], bf16, tag=f"xfbv{t}", name=f"xfbv{t}")
                    for t in range(2)]
            for t in range(2):
                nc.vector.tensor_scalar_add(
                    out=xfbv[t], in0=xfb[t][:, 0:_NQ], scalar1=bv2[:, t:t + 1])

            # ---------------- projections ----------------
            K4 = persist.tile([128, _N], bf16, tag="K4")
            QT4 = persist.tile([128, _NQ], bf16, tag="QT4")
            vt = persist.tile([128, NT_M * _C], vt_dt, tag="vt")

            with tc.tile_pool(name="ps_proj", bufs=4, space="PSUM") as ps_proj:
                def kq_proj(ch, w4, bias, dst):
                    # one 512-col chunk of the 4x-replicated [32,N] projection
                    ps = ps_proj.tile([128, 512], f32, tag="pp", name="pp")
                    for cp in range(2):
                        nc.tensor.matmul(
                            ps,
                            lhsT=w4[cp],
                            rhs=xfb[cp][:, 512 * ch:512 * (ch + 1)],
                            start=(cp == 0), stop=(cp == 1),
                        )
                    nc.scalar.activation(
                        out=dst[:, 512 * ch:512 * (ch + 1)], in_=ps,
                        func=IDENT, bias=bias)

                def v_proj2(nt2):
                    # two key tiles' VT [128, 256] slabs -> one [128,512] bank
                    ps = ps_proj.tile([128, 512], f32, tag="ppv", name="ppv")
                    for j in range(2):
                        nt = 2 * nt2 + j
                        for cp in range(2):
                            nc.tensor.matmul(
                                ps[:, 256 * j:256 * (j + 1)],
                                lhsT=xfb[cp][:, 128 * nt:128 * (nt + 1)],
                                rhs=wvT[cp],
                                start=(cp == 0), stop=(cp == 1),
                            )
                    if nt2 % 2 == 0:
                        nc.scalar.activation(
                            out=vt[:, 512 * nt2:512 * (nt2 + 1)], in_=ps,
                            func=COPY)
                    else:
                        nc.vector.tensor_copy(
                            out=vt[:, 512 * nt2:512 * (nt2 + 1)], in_=ps)

                # interleaved: each 512-wide slice of x feeds its K, Q and V
                # projections while the next slice's DMA is in flight
                for ch in range(_N // 512):
                    kq_proj(ch, wk4, bk4, K4)
                    if ch < NCH_Q:
                        kq_proj(ch, wq4, bq4, QT4)
                    for nt2 in range(2 * ch, 2 * (ch + 1)):
                        v_proj2(nt2)

            # ---------------- attention main loop ----------------
            with tc.tile_pool(name="ps_s", bufs=1, space="PSUM") as ps_s, \
                 tc.tile_pool(name="ps_o", bufs=1, space="PSUM") as ps_o, \
                 tc.tile_pool(name="ps_r", bufs=1, space="PSUM") as ps_r, \
                 tc.tile_pool(name="ps_b", bufs=1, space="PSUM") as ps_b:

                def vt_pair_ap(mp, ct):
                    # lhsT [128, 2, 128] for DoubleRow: key tiles (2mp, 2mp+1)
                    sl = vt[:, 512 * mp + 128 * ct:512 * mp + 128 * ct + 384]
                    return bass.AP(
                        tensor=sl.tensor, offset=sl.offset,
                        ap=[list(sl.ap[0]), [256, 2], [1, 128]])

                def emit_av_half(es, mg, half, out_ps, rs_ps):
                    """A*V + row-sum matmuls for one half of an exp group."""
                    if es_fp8:
                        p = half  # pair index within the es tile
                        mp = 2 * mg + p  # global key-tile-pair index
                        rhs = es[:, 1024 * p:1024 * (p + 1)].rearrange(
                            "p (a b) -> p a b", a=2)
                        for ct in range(2):
                            nc.tensor.matmul(
                                out_ps[:, 512 * ct:512 * (ct + 1)],
                                lhsT=vt_pair_ap(mp, ct),
                                rhs=rhs,
                                start=(mp == 0), stop=(mp == NT_M // 2 - 1),
                                perf_mode=DR,
                                skip_group_check=True,
                            )
                        osl = ones2[:, 0:32]
                        ones_dr = bass.AP(
                            tensor=osl.tensor, offset=osl.offset,
                            ap=[list(osl.ap[0]), [16, 2], [1, 1]])
                        nc.tensor.matmul(
                            rs_ps[0:1, :],
                            lhsT=ones_dr,
                            rhs=rhs,
                            start=(mp == 0), stop=(mp == NT_M // 2 - 1),
                            perf_mode=DR,
                            skip_group_check=True,
                        )
                    else:
                        for g in (2 * half, 2 * half + 1):
                            mt = 4 * mg + g
                            for ct in range(2):
                                nc.tensor.matmul(
                                    out_ps[:, 512 * ct:512 * (ct + 1)],
                                    lhsT=vt[:, _C * mt + 128 * ct:_C * mt + 128 * (ct + 1)],
                                    rhs=es[:, 512 * g:512 * (g + 1)],
                                    start=(mt == 0), stop=(mt == NT_M - 1),
                                    skip_group_check=True,
                                )
                            # col-packed row-sum replicas [32, 512] at group g
                            # (each col group accumulates once per mg: its
                            # start/stop must key on mg, not mt)
                            nc.tensor.matmul(
                                rs_ps[32 * g:32 * (g + 1), :],
                                lhsT=ones32,
                                rhs=es[:, 512 * g:512 * (g + 1)],
                                start=(mg == 0), stop=(mg == NMG - 1),
                                tile_position=(0, 32 * g),
                                skip_group_check=True,
                            )

                if not es_fp8:
                    ones32 = persist.tile([128, 32], bf16, tag="ones32")
                    nc.vector.memset(ones32, 1.0)

                def finalize(out_ps, rs_ps, qc):
                    """normalize + residual + store for one finished chunk"""
                    bc_ps = ps_b.tile([128, 512], f32, tag="bc_ps", name="bc_ps")
                    if es_fp8:
                        # rs_ps [1, 512] totals -> recip -> broadcast matmul
                        recip = outp.tile([1, 512], f32, tag="recip", name="recip")
                        rscr = outp.tile([1, 512], f32, tag="rscr", name="rscr")
                        rcl = outp.tile([1, 512], f32, tag="rcl", name="rcl")
                        nc.vector.tensor_scalar_max(
                            out=rcl, in0=rs_ps[0:1, :], scalar1=1e-20)
                        nc.vector.reciprocal_approx_accurate(
                            out=recip, in_=rcl, scratch=rscr)
                        recb = outp.tile([1, 512], bf16, tag="recb", name="recb")
                        nc.vector.tensor_copy(out=recb, in_=recip)
                        nc.tensor.matmul(
                            bc_ps, lhsT=ones_row, rhs=recb,
                            start=True, stop=True)
                        bc = outp.tile([128, 512], f32, tag="bc", name="bc")
                        nc.vector.tensor_copy(out=bc, in_=bc_ps)
                    else:
                        # rs_ps [128,512] group replicas -> total via sel4
                        rs4 = outp.tile([128, 512], bf16, tag="rs4", name="rs4")
                        nc.vector.tensor_copy(out=rs4, in_=rs_ps)
                        nc.tensor.matmul(bc_ps, lhsT=sel4, rhs=rs4,
                                         start=True, stop=True)
                        bc = outp.tile([128, 512], f32, tag="bc", name="bc")
                        bscr = outp.tile([128, 512], f32, tag="bscr", name="bscr")
                        nc.vector.reciprocal_approx_accurate(
                            out=bc, in_=bc_ps, scratch=bscr)
                    for ct in range(2):
                        o1 = outp.tile([128, 512], bf16, tag="o1", name="o1")
                        o2 = outp.tile([128, 512], bf16, tag="o2", name="o2")
                        nc.vector.tensor_mul(o1, out_ps[:, 512 * ct:512 * (ct + 1)], bc)
                        nc.vector.tensor_add(o2, o1, xfbv[ct][:, 512 * qc:512 * (qc + 1)])
                        nc.gpsimd.dma_start(
                            out=out_d[128 * ct:128 * (ct + 1), 512 * qc:512 * (qc + 1)],
                            in_=o2,
                        )

                # Pipeline: the A*V for exp half h of step i is emitted just
                # before the same-half score matmuls of step i+1, so the PE
                # has work while the ScalarE runs exp, and score matmuls for
                # a half of s_ps wait only on that half's exp (subtile deps).
                # two persistent half-size score tiles: WAR hazards are
                # tracked whole-tile, so each exp half must have its own
                # tile for the next step's score matmuls to overlap it
                s_half = [ps_s.tile([128, 1024], f32, tag=f"s_ps{h}",
                                    name=f"s_ps{h}") for h in range(2)]
                pending = None   # (es, mg, accum) awaiting A*V emission
                fin = None       # (out_ps, rs_ps, qc) awaiting finalize
                for qc in range(NCH_Q):
                    cur = None
                    for mg in range(NMG):
                        for half in range(2):
                            if pending is not None:
                                emit_av_half(pending[0], pending[1], half,
                                             *pending[2])
                            for g in (2 * half, 2 * half + 1):
                                mt = 4 * mg + g
                                gg = g - 2 * half
                                nc.tensor.matmul(
                                    s_half[half][:, 512 * gg:512 * (gg + 1)],
                                    lhsT=K4[32 * g:32 * (g + 1), 128 * mt:128 * (mt + 1)],
                                    rhs=QT4[32 * g:32 * (g + 1), 512 * qc:512 * (qc + 1)],
                                    start=True, stop=True,
                                    tile_position=(32 * g, 0),
                                )
                        es = expp.tile([128, 2048], es_dt, tag="es", name="es")
                        for h in range(2):
                            nc.scalar.activation(
                                out=es[:, 1024 * h:1024 * (h + 1)],
                                in_=s_half[h],
                                func=EXP, bias=nshift)
                        if fin is not None:
                            finalize(*fin)
                            fin = None
                        if cur is None:
                            # allocated only after the previous chunk's
                            # finalize is emitted (bufs=1 bank reuse)
                            cur = (ps_o.tile([128, 1024], f32,
                                             tag="out_ps", name="out_ps"),
                                   ps_r.tile([128, 512], f32,
                                             tag="rs_ps", name="rs_ps"))
                        pending = (es, mg, cur)
                    # tail: emit the last group's A*V now; finalize next chunk
                    for half in range(2):
                        emit_av_half(pending[0], pending[1], half, *pending[2])
                    pending = None
                    fin = (cur[0], cur[1], qc)
                finalize(*fin)

    nc.compile()
    return nc


def _make_runner(nc):
    """Cached jitted SPMD executor, mirroring bass2jax.run_bass_via_pjrt."""
    import jax
    from jax.experimental.shard_map import shard_map
    from jax.sharding import Mesh, PartitionSpec
    from concourse import mybir
    from concourse.bass2jax import (
        _bass_exec_p,
        install_neuronx_cc_hook,
        partition_id_tensor,
    )

    install_neuronx_cc_hook()

    partition_name = (
        nc.partition_id_tensor.name if nc.partition_id_tensor else None)
    in_names, out_names, out_avals = [], [], []
    for alloc in nc.m.functions[0].allocations:
        if not isinstance(alloc, mybir.MemoryLocationSet):
            continue
        name = alloc.memorylocations[0].name
        if alloc.kind == "ExternalInput":
            if name != partition_name:
                in_names.append(name)
        elif alloc.kind == "ExternalOutput":
            out_names.append(name)
            out_avals.append(
                jax.core.ShapedArray(tuple(alloc.tensor_shape),
                                     mybir.dt.np(alloc.dtype)))
    n_params = len(in_names)
    all_in_names = tuple(in_names + out_names)
    if partition_name is not None:
        all_in_names = all_in_names + (partition_name,)

    def _body(*args):
        operands = list(args)
        if partition_name is not None:
            operands.append(partition_id_tensor())
        outs = _bass_exec_p.bind(
            *operands,
            out_avals=tuple(out_avals),
            in_names=all_in_names,
            out_names=tuple(out_names),
            lowering_input_output_aliases=(),
            sim_require_finite=True,
            sim_require_nnan=True,
            nc=nc,
        )
        return tuple(outs)

    devices = jax.devices()[:_NCORES]
    assert len(devices) == _NCORES
    mesh = Mesh(np.asarray(devices), ("core",))
    n_outs = len(out_names)
    in_specs = (PartitionSpec("core"),) * (n_params + n_outs)
    out_specs = (PartitionSpec("core"),) * n_outs
    donate = tuple(range(n_params, n_params + n_outs))
    sharded = jax.jit(
        shard_map(_body, mesh=mesh, in_specs=in_specs, out_specs=out_specs,
                  check_rep=False),
        donate_argnums=donate, keep_unused=True)
    return {
        "fn": sharded,
        "in_names": in_names,
        "out_names": out_names,
        "out_avals": out_avals,
    }


def _get_runtime():
    if "runner" not in _RT:
        _ensure_imports()
        nc = _build_nc()
        _RT["nc"] = nc
        _RT["runner"] = _make_runner(nc)
    return _RT["runner"]


def _core_inputs(x, wq, bq, wk, bk, wv, bv):
    """Build the 8 per-core input dicts (host-side shard)."""
    x = np.ascontiguousarray(np.asarray(x, dtype=np.float32))
    wq = np.asarray(wq, dtype=np.float32)
    bq = np.asarray(bq, dtype=np.float32)
    wk = np.asarray(wk, dtype=np.float32)
    bk = np.asarray(bk, dtype=np.float32)
    wv = np.asarray(wv, dtype=np.float32)
    bv = np.asarray(bv, dtype=np.float32)

    wkq4 = np.concatenate(
        [np.tile(wk.T, (1, 4)), np.tile(wq.T, (1, 4))], axis=1).astype(_BF16)
    wvT = np.ascontiguousarray(wv.T).astype(_BF16)
    bias4 = np.stack(
        [np.tile(bq, 4), np.tile(bk, 4),
         bv[0:128], bv[128:256]], axis=1).astype(np.float32)
    bias4 = np.ascontiguousarray(bias4)
    sel4 = np.zeros((128, 128), dtype=_BF16)
    sel4[[0, 32, 64, 96], :] = 1

    in_maps = []
    for c in range(_NCORES):
        b, h = divmod(c, 2)
        xb = x[b].reshape(_C, _N)
        if h:
            xrot = np.roll(xb, -_NQ, axis=1)
        else:
            xrot = xb
        in_maps.append({
            "xf": np.ascontiguousarray(xrot).astype(_BF16),
            "wkq4": wkq4, "wvT": wvT, "bias4": bias4, "sel4": sel4,
        })
    return in_maps


def run_cores(in_maps):
    """Execute the SPMD kernel; returns list of per-core output dicts."""
    r = _get_runtime()
    fn, in_names, out_names, out_avals = (
        r["fn"], r["in_names"], r["out_names"], r["out_avals"])
    per_core = [[np.asarray(m[n]) for n in in_names] for m in in_maps]
    concat_in = [
        np.concatenate([per_core[c][i] for c in range(_NCORES)], axis=0)
        for i in range(len(in_names))
    ]
    concat_zeros = [
        np.zeros((_NCORES * a.shape[0], *a.shape[1:]), a.dtype)
        for a in out_avals
    ]
    out_arrs = fn(*concat_in, *concat_zeros)
    return [
        {
            name: np.asarray(out_arrs[i]).reshape(_NCORES, *out_avals[i].shape)[c]
            for i, name in enumerate(out_names)
        }
        for c in range(_NCORES)
    ]


def kernel(x, wq, bq, wk, bk, wv, bv):
    x = np.asarray(x, dtype=np.float32)
    B, C, D, H, W = x.shape
    assert (B, C, D * H * W) == (_B, _C, _N)

    in_maps = _core_inputs(x, wq, bq, wk, bk, wv, bv)
    results = run_cores(in_maps)

    out = np.empty((_B, _C, _N), dtype=np.float32)
    for c in range(_NCORES):
        b, h = divmod(c, 2)
        out[b][:, h * _NQ:(h + 1) * _NQ] = results[c]["out"]
    return out.reshape(B, C, D, H, W)
